# revision 52
# baseline (speedup 1.0000x reference)
"""TRN2 Bass kernel for nn_Attention_87308095193383.

Sharding: 8 cores = (batch b in 0..3) x (query-half h in 0..1).
Each core computes, for its batch:
  - conv1/conv2 + GroupNorm fully (stats need full N),
  - pe_attn^T slice [m=2048, n=1024] for its query half,
  - k,v fully; q for its half,
  - attention + proj for its half -> outT [512, 1024].
Host permutes the N columns per core so "my queries" are always columns
0:1024 of the device program (SPMD: one program, per-core data).

Perf notes (cost model: matmul time = out_free * pe_cycle * cyc_per_row):
  - conv and qkv matmuls run as fp8e4m3 error-feedback pairs (A ~ A8+Ar,
    host-split) with DoubleRow perf mode: 3 passes of 0.5 cyc/row beat
    one bf16 pass (1 cyc/row) at the same ~2e-3 accuracy. z/s/u/proj
    stay bf16: fp8 on the attention chain fails the 2e-2 gate because
    the output is a near-cancelling weighted average (quantization noise
    does not shrink relative to the sqrt(N)-suppressed signal).
  - Weights are scaled x16 on host so fp8 values sit in the normal range;
    GroupNorm is scale-invariant (eps -> 256*eps, conv bias x16), the
    qk logits pick up x256 folded into the exp scale, and v's x16 cancels
    against a 16-valued denominator column (VW=66: DoubleRow-legal even
    width).
  - Engine split (GPSIMD cannot touch PSUM on real hw; DMA cannot read
    PSUM): exp/sigmoid and part of the PSUM evacuation on Act; s*pa muls
    mostly DVE (only engine with PSUM TensorTensor); 3/8 of muls route
    via an Act PSUM->SBUF copy so Pool can legally multiply in SBUF.
  - Attention streams are nq-major with proj inlined per query chunk so
    the projection tail hides inside the attention phase.
"""
import numpy as np
import ml_dtypes

import concourse.bass as bass
import concourse.mybir as mybir
import concourse.tile as tile
from concourse import bacc
from concourse.bass_utils import run_bass_kernel_spmd

F32R = mybir.dt.float32r
F32 = mybir.dt.float32
BF16 = mybir.dt.bfloat16
F8 = mybir.dt.float8e4
AF = mybir.ActivationFunctionType
ALU = mybir.AluOpType
DR = mybir.MatmulPerfMode.DoubleRow

N_CORES = 8
C = 512          # channels
CT = C // 128    # 4 c-tiles
N = 2048         # sequence length
NT = N // 128    # 16 m-tiles
NQ = 1024        # queries per core
H = 8            # heads
D = 64           # head dim
SCALE = D ** -0.5
EPS = 1e-5
WS = 16.0        # host weight scale (fp8 range)
GROUPS = 8       # 2 groups per 128-partition tile (64 ch/group)

# mt2 slots routed via Act copy (PSUM->SBUF bf16) + Pool mul (SBUF only;
# GPSIMD cannot access PSUM on real hw). Other slots: DVE mul from PSUM.
POOL_PATH_A = (2,)
POOL_PATH_B = (2, 6)
VW = 66          # v row width: 64 d + denominator col + pad (DR needs even)


def build():
    nc = bacc.Bacc("TRN2", target_bir_lowering=False, debug=False,
                   num_devices=N_CORES)

    def din(name, shape, dt):
        return nc.dram_tensor(name, shape, dt, kind="ExternalInput").ap()

    peT = [din(f"peT{i}", [C, N], F8) for i in range(2)]
    xT = [din(f"xT{i}", [C, N], F8) for i in range(2)]
    cw1 = [din(f"cw1_{i}", [C, C], F8) for i in range(2)]   # conv1_w.T*16
    cw2 = [din(f"cw2_{i}", [C, C], F8) for i in range(2)]
    qw = [din(f"qw{i}", [C, 3 * C], F8) for i in range(2)]  # qkv_w.T*16
    pw = din("pw", [C, C], BF16)    # proj_w.T
    cb1 = din("cb1", [C], F32)      # conv1_b * 16
    cb2 = din("cb2", [C], F32)
    gn1g = din("gn1g", [C], F32)
    gn1b = din("gn1b", [C], F32)
    gn2g = din("gn2g", [C], F32)
    gn2b = din("gn2b", [C], F32)
    pb = din("pb", [C], F32)
    gmask_in = din("gmask", [128, 2], F32)
    gmaskT_in = din("gmaskT", [2, 128], F32)
    outT = nc.dram_tensor("outT", [C, NQ], F32, kind="ExternalOutput").ap()

    with tile.TileContext(nc) as tc:
        _build_body(nc, tc, peT, xT, cw1, cw2, qw, pw, cb1, cb2,
                    gn1g, gn1b, gn2g, gn2b, pb, gmask_in, gmaskT_in, outT)
    nc.compile()
    return nc


def _build_body(nc, tc, peT, xT, cw1, cw2, qw, pw, cb1, cb2,
                gn1g, gn1b, gn2g, gn2b, pb, gmask_in, gmaskT_in, outT):
    from contextlib import ExitStack
    ctx = ExitStack()
    with ctx:
        consts = ctx.enter_context(tc.tile_pool(name="consts", bufs=1))
        work = ctx.enter_context(tc.tile_pool(name="work", bufs=3))
        dma_engs = (nc.sync, nc.scalar, nc.gpsimd)

        # long-lived pools first (LIFO) so the inputs pool can be freed
        # after qkv, letting p1/p2 stay resident through attention.
        pa_pool = ctx.enter_context(tc.tile_pool(name="pa", bufs=1))
        pa = pa_pool.tile([128, NT, NQ], BF16)     # (tanh(z/2)) tiles
        kqv_pool = ctx.enter_context(tc.tile_pool(name="kqv", bufs=1))
        kT_sb = kqv_pool.tile([128, CT, N], BF16)
        qT_sb = kqv_pool.tile([128, CT, NQ], BF16)
        v_sb = kqv_pool.tile([128, NT, H, VW], BF16)
        nc.vector.memset(v_sb[:, :, :, D:VW], WS)
        p12_pool = ctx.enter_context(tc.tile_pool(name="p12", bufs=1))
        p1_sb = p12_pool.tile([128, CT, NQ], BF16)
        p2_sb = p12_pool.tile([128, CT, N], BF16)
        pwp = ctx.enter_context(tc.tile_pool(name="pwp", bufs=1))
        pw_sb = pwp.tile([128, CT, C], BF16)
        nc.sync.dma_start(pw_sb, pw.rearrange("(t p) o -> p t o", p=128))

        # ---- prefetch every DRAM input up front (DMA idles later anyway)
        in_ctx = tc.tile_pool(name="inp", bufs=1)
        in_pool = in_ctx.__enter__()
        pe_sb = [in_pool.tile([128, CT, N], F8, name=f"pe_sb{i}")
                 for i in range(2)]
        x_sb = [in_pool.tile([128, CT, N], F8, name=f"x_sb{i}")
                 for i in range(2)]
        qw_sb = [in_pool.tile([128, CT, 3 * C], F8, name=f"qw_sb{i}")
                 for i in range(2)]
        cw1_sb = [in_pool.tile([128, CT, C], F8, name=f"cw1_sb{i}")
                  for i in range(2)]
        cw2_sb = [in_pool.tile([128, CT, C], F8, name=f"cw2_sb{i}")
                  for i in range(2)]
        # conv inputs (pe + cw) first so conv matmuls start ASAP; x/qw
        # (needed only by qkv, ~40us later) queue behind them.
        for i in range(2):
            pe_r = peT[i].rearrange("(t p) n -> p t n", p=128)
            for ct in range(CT):
                dma_engs[(ct + i) % 3].dma_start(pe_sb[i][:, ct], pe_r[:, ct])
            nc.gpsimd.dma_start(cw1_sb[i],
                                cw1[i].rearrange("(t p) o -> p t o", p=128))
            nc.scalar.dma_start(cw2_sb[i],
                                cw2[i].rearrange("(t p) o -> p t o", p=128))
        for i in range(2):
            x_r = xT[i].rearrange("(t p) n -> p t n", p=128)
            qw_r = qw[i].rearrange("(t p) o -> p t o", p=128)
            for ct in range(CT):
                dma_engs[(ct + 1 + i) % 3].dma_start(x_sb[i][:, ct], x_r[:, ct])
                dma_engs[(ct + 2 + i) % 3].dma_start(qw_sb[i][:, ct], qw_r[:, ct])
        # ---- constants
        gmask = consts.tile([128, 2], F32)     # group-membership mask
        nc.sync.dma_start(gmask, gmask_in)
        gmaskT = consts.tile([2, 128], F32)
        nc.sync.dma_start(gmaskT, gmaskT_in)
        epst = consts.tile([128, 1], F32)
        nc.vector.memset(epst, EPS * WS * WS)   # eps in the x16 domain

        bias1 = consts.tile([128, CT], F32)
        nc.sync.dma_start(bias1, cb1.rearrange("(t p) -> p t", p=128))
        bias2 = consts.tile([128, CT], F32)
        nc.sync.dma_start(bias2, cb2.rearrange("(t p) -> p t", p=128))
        g1g = consts.tile([128, CT], F32)
        nc.sync.dma_start(g1g, gn1g.rearrange("(t p) -> p t", p=128))
        g1b = consts.tile([128, CT], F32)
        nc.sync.dma_start(g1b, gn1b.rearrange("(t p) -> p t", p=128))
        g2g = consts.tile([128, CT], F32)
        nc.sync.dma_start(g2g, gn2g.rearrange("(t p) -> p t", p=128))
        g2b = consts.tile([128, CT], F32)
        nc.sync.dma_start(g2b, gn2b.rearrange("(t p) -> p t", p=128))
        pbias = consts.tile([128, CT], F32)
        nc.sync.dma_start(pbias, pb.rearrange("(t p) -> p t", p=128))

        ps_abcd = tc.tile_pool(name="ps_mm", bufs=4, space="PSUM")
        ps_mm = ps_abcd.__enter__()

        # ================= stage A/B: conv + groupnorm =================
        if True:
            for conv_i, (cwsb, cbt, gg, gb, dst8, keep) in enumerate([
                    (cw1_sb, bias1, g1g, g1b, p1_sb, NQ),
                    (cw2_sb, bias2, g2g, g2b, p2_sb, N)]):
                stats = work.tile([128, CT, N // 512, 6], F32, tag="gnstats")
                mv2 = work.tile([128, 2, CT], F32, tag="gnmv")
                stack3 = work.tile([128, 3, CT], F32, tag="gnstack")
                for ot in range(CT):
                    for nch in range(N // 512):
                        ps = ps_mm.tile([128, 512], F32, tag="mm")
                        passes = [(0, 0), (0, 1), (1, 0)]
                        for pi, (wi, ai) in enumerate(passes):
                            for cp in range(CT // 2):
                                nc.tensor.matmul(
                                    ps,
                                    cwsb[wi][:, 2 * cp:2 * cp + 2,
                                             ot * 128:(ot + 1) * 128],
                                    pe_sb[ai][:, 2 * cp:2 * cp + 2,
                                              nch * 512:(nch + 1) * 512],
                                    start=(pi == 0 and cp == 0),
                                    stop=(pi == 2 and cp == CT // 2 - 1),
                                    perf_mode=DR)
                        nc.vector.bn_stats(stats[:, ot, nch], ps)
                        if nch * 512 < keep:
                            nc.scalar.copy(
                                dst8[:, ot, nch * 512:(nch + 1) * 512], ps)
                    nc.vector.bn_aggr(mv2[:, :, ot], stats[:, ot])
                nc.vector.tensor_add(stack3[:, 0], mv2[:, 0], cbt)
                nc.vector.tensor_copy(stack3[:, 1], mv2[:, 1])
                nc.vector.tensor_mul(stack3[:, 2], stack3[:, 0], stack3[:, 0])
                # group sums over 64-partition halves (all ots at once, N=12)
                gs = ps_mm.tile([2, 3, CT], F32, tag="mm")
                nc.tensor.matmul(gs, gmask, stack3.rearrange("p a t -> p (a t)"),
                                 start=True, stop=True)
                gss = work.tile([2, 3, CT], F32, tag="gss")
                nc.scalar.copy(gss, gs)
                gstat = work.tile([2, 2, CT], F32, tag="gstat")  # [mean, rstd]
                nc.vector.tensor_scalar_mul(gstat[:, 0], gss[:, 0], 1.0 / 64.0)
                vt = work.tile([2, 2, CT], F32, tag="gvtmp")
                nc.vector.tensor_add(vt[:, 0], gss[:, 1], gss[:, 2])
                nc.vector.tensor_scalar_mul(vt[:, 0], vt[:, 0], 1.0 / 64.0)
                nc.vector.tensor_mul(vt[:, 1], gstat[:, 0], gstat[:, 0])
                nc.vector.tensor_sub(vt[:, 0], vt[:, 0], vt[:, 1])
                nc.scalar.activation(vt[:, 0], vt[:, 0], AF.Sqrt, bias=epst[0:2])
                nc.vector.reciprocal(gstat[:, 1], vt[:, 0])
                # broadcast group [mean, rstd] to partitions via indicator MM
                bc_ps = ps_mm.tile([128, 2, CT], F32, tag="mm")
                nc.tensor.matmul(bc_ps, gmaskT,
                                 gstat.rearrange("p a t -> p (a t)"),
                                 start=True, stop=True)
                bcst = work.tile([128, 2, CT], F32, tag="gbc")
                nc.scalar.copy(bcst, bc_ps)
                # per-channel affine: y = x*sc + sh  (bf16 staging -> fp8)
                sc = work.tile([128, 2, CT], F32, tag="gsc")
                nc.vector.tensor_mul(sc[:, 0], bcst[:, 1], gg)
                nc.vector.tensor_sub(sc[:, 1], cbt, bcst[:, 0])
                nc.vector.tensor_mul(sc[:, 1], sc[:, 1], sc[:, 0])
                nc.vector.tensor_add(sc[:, 1], sc[:, 1], gb)
                for ot in range(CT):
                    aeng = nc.vector if (conv_i == 1 and ot % 2) else nc.gpsimd
                    aeng.tensor_scalar(
                        dst8[:, ot, 0:keep], dst8[:, ot, 0:keep],
                        sc[:, 0, ot:ot + 1], sc[:, 1, ot:ot + 1],
                        op0=ALU.mult, op1=ALU.add)

            # ================= stage D: qkv =================
            # kT (full N) and qT (first NQ) in bf16; v in fp8 (x16)
            for ot in range(CT):
                for nch in range(N // 512):
                    ps = ps_mm.tile([128, 512], F32, tag="mm")
                    for pi, (wi, ai) in enumerate([(0, 0), (0, 1), (1, 0)]):
                        for cp in range(CT // 2):
                            nc.tensor.matmul(
                                ps,
                                qw_sb[wi][:, 2 * cp:2 * cp + 2,
                                          C + ot * 128:C + (ot + 1) * 128],
                                x_sb[ai][:, 2 * cp:2 * cp + 2,
                                         nch * 512:(nch + 1) * 512],
                                start=(pi == 0 and cp == 0),
                                stop=(pi == 2 and cp == CT // 2 - 1),
                                perf_mode=DR)
                    keng = nc.scalar if nch % 2 else nc.vector
                    (keng.copy if nch % 2 else keng.tensor_copy)(
                        kT_sb[:, ot, nch * 512:(nch + 1) * 512], ps)
                for nch in range(NQ // 512):
                    ps = ps_mm.tile([128, 512], F32, tag="mm")
                    for pi, (wi, ai) in enumerate([(0, 0), (0, 1), (1, 0)]):
                        for cp in range(CT // 2):
                            nc.tensor.matmul(
                                ps,
                                qw_sb[wi][:, 2 * cp:2 * cp + 2,
                                          ot * 128:(ot + 1) * 128],
                                x_sb[ai][:, 2 * cp:2 * cp + 2,
                                         nch * 512:(nch + 1) * 512],
                                start=(pi == 0 and cp == 0),
                                stop=(pi == 2 and cp == CT // 2 - 1),
                                perf_mode=DR)
                    nc.scalar.copy(
                        qT_sb[:, ot, nch * 512:(nch + 1) * 512], ps)
            for nt in range(NT):
                ps = ps_mm.tile([128, 512], F32, tag="mm")
                for pi, (wi, ai) in enumerate([(0, 0), (0, 1), (1, 0)]):
                    for cp in range(CT // 2):
                        nc.tensor.matmul(
                            ps,
                            x_sb[ai][:, 2 * cp:2 * cp + 2,
                                     nt * 128:(nt + 1) * 128],
                            qw_sb[wi][:, 2 * cp:2 * cp + 2, 2 * C:3 * C],
                            start=(pi == 0 and cp == 0),
                            stop=(pi == 2 and cp == CT // 2 - 1),
                            perf_mode=DR)
                nc.vector.tensor_copy(v_sb[:, nt, :, 0:D],
                                      ps.rearrange("p (h d) -> p h d", h=H))

        # ================= stage E: attention =================
        ps_abcd.__exit__(None, None, None)
        in_ctx.__exit__(None, None, None)   # free pe/x/qw/cw SBUF
        out_pool = ctx.enter_context(tc.tile_pool(name="outp", bufs=1))
        t2_pool = ctx.enter_context(tc.tile_pool(name="t2p", bufs=4))
        e2_pool = ctx.enter_context(tc.tile_pool(name="e2p", bufs=4))
        o_sb = out_pool.tile([128, CT, NQ], BF16)

        ps_e = ExitStack()
        ps_t2 = ps_e.enter_context(tc.tile_pool(name="ps_t2", bufs=2,
                                                space="PSUM"))
        ps_u = ps_e.enter_context(tc.tile_pool(name="ps_u", bufs=2,
                                               space="PSUM"))
        ps_z = ps_e.enter_context(tc.tile_pool(name="ps_z", bufs=2,
                                               space="PSUM"))
        pa1_pool = ps_e.enter_context(tc.tile_pool(name="pa1p", bufs=2))
        POOL_MT = {4: (0, 0), 5: (0, 1), 12: (1, 0), 13: (1, 1)}

        fin = out_pool.tile([128, CT, NQ], F32)
        for nq in range(NQ // 512):
            for h in range(H):
                row0 = (h % 2) * 64
                kt = h // 2
                pool_path = POOL_PATH_A if (h + nq) % 2 else POOL_PATH_B
                u = ps_u.tile([VW, 512], F32, tag="u")
                if h == 0:
                    pa1 = pa1_pool.tile([128, 2, 2, 512], BF16, tag="pa1")
                for mt4 in range(NT // 4):
                    if h == 0:
                        # just-in-time pe_attn: z then tanh(z/2) (tanh lives
                        # in the exp act table -> no table thrash)
                        for mt in range(4 * mt4, 4 * mt4 + 4):
                            zp = ps_z.tile([128, 512], F32, tag="z")
                            for ct in range(CT):
                                nc.tensor.matmul(
                                    zp,
                                    p2_sb[:, ct, mt * 128:(mt + 1) * 128],
                                    p1_sb[:, ct, nq * 512:(nq + 1) * 512],
                                    start=(ct == 0), stop=(ct == CT - 1))
                            nc.scalar.activation(
                                pa[:, mt, nq * 512:(nq + 1) * 512], zp,
                                AF.Tanh, scale=0.5)
                            if mt in POOL_MT:
                                si, sj = POOL_MT[mt]
                                nc.vector.tensor_scalar_add(
                                    pa1[:, si, sj],
                                    pa[:, mt, nq * 512:(nq + 1) * 512], 1.0)
                    # two mt2 pairs -> one staged [128, 4, 512] bf16 tile,
                    # one exp instruction, four u-steps
                    t2sb = t2_pool.tile([128, 4, 512], BF16, tag="t2sb")
                    e2 = e2_pool.tile([128, 4, 512], BF16, tag="e2")
                    for half in range(2):
                        mt2 = 2 * mt4 + half
                        t2ps = ps_t2.tile([128, 2, 512], F32, tag="t2")
                        for j in range(2):
                            mt = 2 * mt2 + j
                            nc.tensor.matmul(
                                t2ps[:, j],
                                kT_sb[row0:row0 + 64, kt,
                                      mt * 128:(mt + 1) * 128],
                                qT_sb[row0:row0 + 64, kt,
                                      nq * 512:(nq + 1) * 512],
                                start=True, stop=True)
                        pa_sl = pa[:, 2 * mt2:2 * mt2 + 2,
                                   nq * 512:(nq + 1) * 512]
                        if mt2 % 8 in pool_path:
                            c2 = t2_pool.tile([128, 2, 512], BF16, tag="c2")
                            nc.scalar.copy(c2, t2ps)
                            nc.gpsimd.tensor_mul(
                                t2sb[:, 2 * half:2 * half + 2], c2,
                                pa1[:, 0 if mt2 == 2 else 1])
                        else:
                            nc.vector.scalar_tensor_tensor(
                                t2sb[:, 2 * half:2 * half + 2], pa_sl, 1.0,
                                t2ps, op0=ALU.add, op1=ALU.mult)
                    nc.scalar.activation(e2, t2sb, AF.Exp,
                                         scale=SCALE / (WS * WS) / 2)
                    for j in range(4):
                        mt = 4 * mt4 + j
                        nc.tensor.matmul(
                            u, v_sb[:, mt, h, :], e2[:, j],
                            start=(mt == 0), stop=(mt == NT - 1))
                # o = u[0:D] / den, den broadcast via tiny ones-matmul
                rec = work.tile([1, 512], BF16, tag="rec")
                with nc.allow_low_precision(reason="1/den fits bf16"):
                    nc.vector.reciprocal(rec, u[D:D + 1])
                bc = work.tile([D, 512], BF16, tag="recbc")
                nc.gpsimd.partition_broadcast(bc, rec)
                nc.vector.tensor_mul(
                    o_sb[row0:row0 + 64, kt, nq * 512:(nq + 1) * 512],
                    u[0:D], bc)

            # ---- proj for this query chunk (stage F, inline)
            for ot in range(CT):
                ps = ps_u.tile([128, 512], F32, tag="u")
                for ct in range(CT):
                    nc.tensor.matmul(
                        ps, pw_sb[:, ct, ot * 128:(ot + 1) * 128],
                        o_sb[:, ct, nq * 512:(nq + 1) * 512],
                        start=(ct == 0), stop=(ct == CT - 1))
                nc.vector.tensor_scalar_add(
                    fin[:, ot, nq * 512:(nq + 1) * 512], ps,
                    pbias[:, ot:ot + 1])
                dma_engs[ot % 3].dma_start(
                    outT.rearrange("(t p) n -> p t n",
                                   p=128)[:, ot, nq * 512:(nq + 1) * 512],
                    fin[:, ot, nq * 512:(nq + 1) * 512])
        ps_e.close()


_NC_CACHE = {}


def _get_nc():
    if "nc" not in _NC_CACHE:
        _NC_CACHE["nc"] = build()
    return _NC_CACHE["nc"]


def make_in_maps(x, pe, qkv_w, proj_w, proj_b, conv1_w, conv1_b, gn1_g, gn1_b,
                 conv2_w, conv2_b, gn2_g, gn2_b):
    f = np.float32
    f8 = ml_dtypes.float8_e4m3
    bf = ml_dtypes.bfloat16
    def pair8(a):
        a8 = a.astype(f8)
        ar = (a - a8.astype(f)).astype(f8)
        return a8, ar

    cw1p = pair8(np.asarray(conv1_w, f).T * WS)
    cw2p = pair8(np.asarray(conv2_w, f).T * WS)
    qwp = pair8(np.asarray(qkv_w, f).T * WS)
    shared = {
        "cw1_0": cw1p[0], "cw1_1": cw1p[1],
        "cw2_0": cw2p[0], "cw2_1": cw2p[1],
        "qw0": qwp[0], "qw1": qwp[1],
        "pw": np.asarray(proj_w, f).T.astype(bf),
        "cb1": np.asarray(conv1_b, f) * WS,
        "cb2": np.asarray(conv2_b, f) * WS,
        "gn1g": np.asarray(gn1_g, f),
        "gn1b": np.asarray(gn1_b, f),
        "gn2g": np.asarray(gn2_g, f),
        "gn2b": np.asarray(gn2_b, f),
        "pb": np.asarray(proj_b, f),
        "gmask": np.repeat(np.eye(2, dtype=f), 64, axis=0),
        "gmaskT": np.ascontiguousarray(np.repeat(np.eye(2, dtype=f), 64,
                                                 axis=0).T),
    }
    shared = {k: np.ascontiguousarray(v) for k, v in shared.items()}
    in_maps = []
    for c in range(N_CORES):
        b, h = c // 2, c % 2
        xT = np.asarray(x[b], f).T
        peT = np.asarray(pe[b], f).T
        if h == 1:
            xT = np.concatenate([xT[:, NQ:], xT[:, :NQ]], axis=1)
            peT = np.concatenate([peT[:, NQ:], peT[:, :NQ]], axis=1)
        m = dict(shared)
        xp = pair8(xT)
        pep = pair8(peT)
        m["xT0"], m["xT1"] = xp
        m["peT0"], m["peT1"] = pep
        in_maps.append(m)
    return in_maps


def assemble_out(results):
    B = N_CORES // 2
    out = np.empty((B, N, C), np.float32)
    for c in range(N_CORES):
        b, h = c // 2, c % 2
        out[b, h * NQ:(h + 1) * NQ, :] = results[c]["outT"].T
    return out


def kernel(**inputs):
    nc = _get_nc()
    in_maps = make_in_maps(**inputs)
    r = run_bass_kernel_spmd(nc, in_maps, core_ids=list(range(N_CORES)))
    return assemble_out(r.results)


if __name__ == "__main__":
    nc = build()
    print("build+compile OK")


# revision 53
# speedup vs baseline: 1.0857x; 1.0857x over previous
"""TRN2 Bass kernel for nn_Attention_87308095193383.

Sharding: 8 cores = (batch b in 0..3) x (query-half h in 0..1).
Each core computes, for its batch:
  - conv1/conv2 + GroupNorm fully (stats need full N),
  - pe_attn^T slice [m=2048, n=1024] for its query half,
  - k,v fully; q for its half,
  - attention + proj for its half -> outT [512, 1024].
Host permutes the N columns per core so "my queries" are always columns
0:1024 of the device program (SPMD: one program, per-core data).

Perf notes (cost model: matmul time = out_free * pe_cycle * cyc_per_row):
  - conv and qkv matmuls run as fp8e4m3 error-feedback pairs (A ~ A8+Ar,
    host-split) with DoubleRow perf mode: 3 passes of 0.5 cyc/row beat
    one bf16 pass (1 cyc/row) at the same ~2e-3 accuracy. z/s/u/proj
    stay bf16: fp8 on the attention chain fails the 2e-2 gate because
    the output is a near-cancelling weighted average (quantization noise
    does not shrink relative to the sqrt(N)-suppressed signal).
  - Weights are scaled x16 on host so fp8 values sit in the normal range;
    GroupNorm is scale-invariant (eps -> 256*eps, conv bias x16), the
    qk logits pick up x256 folded into the exp scale, and v's x16 cancels
    against a 16-valued denominator column (VW=66: DoubleRow-legal even
    width).
  - Engine split (GPSIMD cannot touch PSUM on real hw; DMA cannot read
    PSUM): exp/sigmoid and part of the PSUM evacuation on Act; s*pa muls
    mostly DVE (only engine with PSUM TensorTensor); 3/8 of muls route
    via an Act PSUM->SBUF copy so Pool can legally multiply in SBUF.
  - Attention streams are nq-major with proj inlined per query chunk so
    the projection tail hides inside the attention phase.
"""
import numpy as np
import ml_dtypes

import concourse.bass as bass
import concourse.mybir as mybir
import concourse.tile as tile
from concourse import bacc
from concourse.bass_utils import run_bass_kernel_spmd

F32R = mybir.dt.float32r
F32 = mybir.dt.float32
BF16 = mybir.dt.bfloat16
F8 = mybir.dt.float8e4
AF = mybir.ActivationFunctionType
ALU = mybir.AluOpType
DR = mybir.MatmulPerfMode.DoubleRow

N_CORES = 8
C = 512          # channels
CT = C // 128    # 4 c-tiles
N = 2048         # sequence length
NT = N // 128    # 16 m-tiles
NQ = 1024        # queries per core
H = 8            # heads
D = 64           # head dim
SCALE = D ** -0.5
EPS = 1e-5
WS = 16.0        # host weight scale (fp8 range)
GROUPS = 8       # 2 groups per 128-partition tile (64 ch/group)

# mt2 slots routed via Act copy (PSUM->SBUF bf16) + Pool mul (SBUF only;
# GPSIMD cannot access PSUM on real hw). Other slots: DVE mul from PSUM.
POOL_PATH_A = (2,)
POOL_PATH_B = (2, 6)
VW = 66          # v row width: 64 d + denominator col + pad (DR needs even)


def build():
    nc = bacc.Bacc("TRN2", target_bir_lowering=False, debug=False,
                   num_devices=N_CORES)

    def din(name, shape, dt):
        return nc.dram_tensor(name, shape, dt, kind="ExternalInput").ap()

    peT = [din(f"peT{i}", [C, N], F8) for i in range(2)]
    xT = [din(f"xT{i}", [C, N], F8) for i in range(2)]
    cw1 = [din(f"cw1_{i}", [C, C], F8) for i in range(2)]   # conv1_w.T*16
    cw2 = [din(f"cw2_{i}", [C, C], F8) for i in range(2)]
    qw = [din(f"qw{i}", [C, 3 * C], F8) for i in range(2)]  # qkv_w.T*16
    pw = din("pw", [C, C], BF16)    # proj_w.T
    cb1 = din("cb1", [C], F32)      # conv1_b * 16
    cb2 = din("cb2", [C], F32)
    gn1g = din("gn1g", [C], F32)
    gn1b = din("gn1b", [C], F32)
    gn2g = din("gn2g", [C], F32)
    gn2b = din("gn2b", [C], F32)
    pb = din("pb", [C], F32)
    gmask_in = din("gmask", [128, 2], F32)
    gmaskT_in = din("gmaskT", [2, 128], F32)
    outT = nc.dram_tensor("outT", [C, NQ], F32, kind="ExternalOutput").ap()

    with tile.TileContext(nc) as tc:
        _build_body(nc, tc, peT, xT, cw1, cw2, qw, pw, cb1, cb2,
                    gn1g, gn1b, gn2g, gn2b, pb, gmask_in, gmaskT_in, outT)
    nc.compile()
    return nc


def _build_body(nc, tc, peT, xT, cw1, cw2, qw, pw, cb1, cb2,
                gn1g, gn1b, gn2g, gn2b, pb, gmask_in, gmaskT_in, outT):
    from contextlib import ExitStack
    ctx = ExitStack()
    with ctx:
        consts = ctx.enter_context(tc.tile_pool(name="consts", bufs=1))
        work = ctx.enter_context(tc.tile_pool(name="work", bufs=3))
        dma_engs = (nc.sync, nc.scalar, nc.gpsimd)

        # ---- prefetch every DRAM input up front (DMA idles later anyway)
        in_pool = ctx.enter_context(tc.tile_pool(name="inp", bufs=1))
        pe_sb = [in_pool.tile([128, CT, N], F8, name=f"pe_sb{i}")
                 for i in range(2)]
        x_sb = [in_pool.tile([128, CT, N], F8, name=f"x_sb{i}")
                 for i in range(2)]
        qw_sb = [in_pool.tile([128, CT, 3 * C], F8, name=f"qw_sb{i}")
                 for i in range(2)]
        cw1_sb = [in_pool.tile([128, CT, C], F8, name=f"cw1_sb{i}")
                  for i in range(2)]
        cw2_sb = [in_pool.tile([128, CT, C], F8, name=f"cw2_sb{i}")
                  for i in range(2)]
        # conv inputs (pe + cw) first so conv matmuls start ASAP; x/qw
        # (needed only by qkv, ~40us later) queue behind them.
        for i in range(2):
            pe_r = peT[i].rearrange("(t p) n -> p t n", p=128)
            for ct in range(CT):
                dma_engs[(ct + i) % 3].dma_start(pe_sb[i][:, ct], pe_r[:, ct])
            nc.gpsimd.dma_start(cw1_sb[i],
                                cw1[i].rearrange("(t p) o -> p t o", p=128))
            nc.scalar.dma_start(cw2_sb[i],
                                cw2[i].rearrange("(t p) o -> p t o", p=128))
        for i in range(2):
            x_r = xT[i].rearrange("(t p) n -> p t n", p=128)
            qw_r = qw[i].rearrange("(t p) o -> p t o", p=128)
            for ct in range(CT):
                dma_engs[(ct + 1 + i) % 3].dma_start(x_sb[i][:, ct], x_r[:, ct])
                dma_engs[(ct + 2 + i) % 3].dma_start(qw_sb[i][:, ct], qw_r[:, ct])
        pw_sb = in_pool.tile([128, CT, C], BF16)
        nc.sync.dma_start(pw_sb, pw.rearrange("(t p) o -> p t o", p=128))

        # ---- constants
        gmask = consts.tile([128, 2], F32)     # group-membership mask
        nc.sync.dma_start(gmask, gmask_in)
        gmaskT = consts.tile([2, 128], F32)
        nc.sync.dma_start(gmaskT, gmaskT_in)
        epst = consts.tile([128, 1], F32)
        nc.vector.memset(epst, EPS * WS * WS)   # eps in the x16 domain

        bias1 = consts.tile([128, CT], F32)
        nc.sync.dma_start(bias1, cb1.rearrange("(t p) -> p t", p=128))
        bias2 = consts.tile([128, CT], F32)
        nc.sync.dma_start(bias2, cb2.rearrange("(t p) -> p t", p=128))
        g1g = consts.tile([128, CT], F32)
        nc.sync.dma_start(g1g, gn1g.rearrange("(t p) -> p t", p=128))
        g1b = consts.tile([128, CT], F32)
        nc.sync.dma_start(g1b, gn1b.rearrange("(t p) -> p t", p=128))
        g2g = consts.tile([128, CT], F32)
        nc.sync.dma_start(g2g, gn2g.rearrange("(t p) -> p t", p=128))
        g2b = consts.tile([128, CT], F32)
        nc.sync.dma_start(g2b, gn2b.rearrange("(t p) -> p t", p=128))
        pbias = consts.tile([128, CT], F32)
        nc.sync.dma_start(pbias, pb.rearrange("(t p) -> p t", p=128))

        # ---- persistent activations
        pa_pool = ctx.enter_context(tc.tile_pool(name="pa", bufs=1))
        pa = pa_pool.tile([128, NT, NQ], BF16)     # sigmoid(pe_attn)^T tiles
        kqv_pool = ctx.enter_context(tc.tile_pool(name="kqv", bufs=1))
        kT_sb = kqv_pool.tile([128, CT, N], BF16)
        qT_sb = kqv_pool.tile([128, CT, NQ], BF16)
        v_sb = kqv_pool.tile([128, NT, H, VW], BF16)
        # denominator column: v's x16 scale cancels against this 16.
        nc.vector.memset(v_sb[:, :, :, D:VW], WS)

        ps_abcd = tc.tile_pool(name="ps_mm", bufs=4, space="PSUM")
        ps_mm = ps_abcd.__enter__()

        # ================= stage A/B: conv + groupnorm =================
        with tc.tile_pool(name="p12", bufs=1) as p12_pool:
            # fp8 staging, GN affine applied in place (scale-preserving).
            p1_sb = p12_pool.tile([128, CT, NQ], BF16)
            p2_sb = p12_pool.tile([128, CT, N], BF16)

            for conv_i, (cwsb, cbt, gg, gb, dst8, keep) in enumerate([
                    (cw1_sb, bias1, g1g, g1b, p1_sb, NQ),
                    (cw2_sb, bias2, g2g, g2b, p2_sb, N)]):
                stats = work.tile([128, CT, N // 512, 6], F32, tag="gnstats")
                mv2 = work.tile([128, 2, CT], F32, tag="gnmv")
                stack3 = work.tile([128, 3, CT], F32, tag="gnstack")
                for ot in range(CT):
                    for nch in range(N // 512):
                        ps = ps_mm.tile([128, 512], F32, tag="mm")
                        passes = [(0, 0), (0, 1), (1, 0)]
                        for pi, (wi, ai) in enumerate(passes):
                            for cp in range(CT // 2):
                                nc.tensor.matmul(
                                    ps,
                                    cwsb[wi][:, 2 * cp:2 * cp + 2,
                                             ot * 128:(ot + 1) * 128],
                                    pe_sb[ai][:, 2 * cp:2 * cp + 2,
                                              nch * 512:(nch + 1) * 512],
                                    start=(pi == 0 and cp == 0),
                                    stop=(pi == 2 and cp == CT // 2 - 1),
                                    perf_mode=DR)
                        nc.vector.bn_stats(stats[:, ot, nch], ps)
                        if nch * 512 < keep:
                            nc.scalar.copy(
                                dst8[:, ot, nch * 512:(nch + 1) * 512], ps)
                    nc.vector.bn_aggr(mv2[:, :, ot], stats[:, ot])
                nc.vector.tensor_add(stack3[:, 0], mv2[:, 0], cbt)
                nc.vector.tensor_copy(stack3[:, 1], mv2[:, 1])
                nc.vector.tensor_mul(stack3[:, 2], stack3[:, 0], stack3[:, 0])
                # group sums over 64-partition halves (all ots at once, N=12)
                gs = ps_mm.tile([2, 3, CT], F32, tag="mm")
                nc.tensor.matmul(gs, gmask, stack3.rearrange("p a t -> p (a t)"),
                                 start=True, stop=True)
                gss = work.tile([2, 3, CT], F32, tag="gss")
                nc.scalar.copy(gss, gs)
                gstat = work.tile([2, 2, CT], F32, tag="gstat")  # [mean, rstd]
                nc.vector.tensor_scalar_mul(gstat[:, 0], gss[:, 0], 1.0 / 64.0)
                vt = work.tile([2, 2, CT], F32, tag="gvtmp")
                nc.vector.tensor_add(vt[:, 0], gss[:, 1], gss[:, 2])
                nc.vector.tensor_scalar_mul(vt[:, 0], vt[:, 0], 1.0 / 64.0)
                nc.vector.tensor_mul(vt[:, 1], gstat[:, 0], gstat[:, 0])
                nc.vector.tensor_sub(vt[:, 0], vt[:, 0], vt[:, 1])
                nc.scalar.activation(vt[:, 0], vt[:, 0], AF.Sqrt, bias=epst[0:2])
                nc.vector.reciprocal(gstat[:, 1], vt[:, 0])
                # broadcast group [mean, rstd] to partitions via indicator MM
                bc_ps = ps_mm.tile([128, 2, CT], F32, tag="mm")
                nc.tensor.matmul(bc_ps, gmaskT,
                                 gstat.rearrange("p a t -> p (a t)"),
                                 start=True, stop=True)
                bcst = work.tile([128, 2, CT], F32, tag="gbc")
                nc.scalar.copy(bcst, bc_ps)
                # per-channel affine: y = x*sc + sh  (bf16 staging -> fp8)
                sc = work.tile([128, 2, CT], F32, tag="gsc")
                nc.vector.tensor_mul(sc[:, 0], bcst[:, 1], gg)
                nc.vector.tensor_sub(sc[:, 1], cbt, bcst[:, 0])
                nc.vector.tensor_mul(sc[:, 1], sc[:, 1], sc[:, 0])
                nc.vector.tensor_add(sc[:, 1], sc[:, 1], gb)
                for ot in range(CT):
                    aeng = nc.vector if (conv_i == 1 and ot % 2) else nc.gpsimd
                    aeng.tensor_scalar(
                        dst8[:, ot, 0:keep], dst8[:, ot, 0:keep],
                        sc[:, 0, ot:ot + 1], sc[:, 1, ot:ot + 1],
                        op0=ALU.mult, op1=ALU.add)

            # ================= stage D: qkv =================
            # kT (full N) and qT (first NQ) in bf16; v in fp8 (x16)
            for ot in range(CT):
                for nch in range(N // 512):
                    ps = ps_mm.tile([128, 512], F32, tag="mm")
                    for pi, (wi, ai) in enumerate([(0, 0), (0, 1), (1, 0)]):
                        for cp in range(CT // 2):
                            nc.tensor.matmul(
                                ps,
                                qw_sb[wi][:, 2 * cp:2 * cp + 2,
                                          C + ot * 128:C + (ot + 1) * 128],
                                x_sb[ai][:, 2 * cp:2 * cp + 2,
                                         nch * 512:(nch + 1) * 512],
                                start=(pi == 0 and cp == 0),
                                stop=(pi == 2 and cp == CT // 2 - 1),
                                perf_mode=DR)
                    keng = nc.scalar if nch % 2 else nc.vector
                    (keng.copy if nch % 2 else keng.tensor_copy)(
                        kT_sb[:, ot, nch * 512:(nch + 1) * 512], ps)
                for nch in range(NQ // 512):
                    ps = ps_mm.tile([128, 512], F32, tag="mm")
                    for pi, (wi, ai) in enumerate([(0, 0), (0, 1), (1, 0)]):
                        for cp in range(CT // 2):
                            nc.tensor.matmul(
                                ps,
                                qw_sb[wi][:, 2 * cp:2 * cp + 2,
                                          ot * 128:(ot + 1) * 128],
                                x_sb[ai][:, 2 * cp:2 * cp + 2,
                                         nch * 512:(nch + 1) * 512],
                                start=(pi == 0 and cp == 0),
                                stop=(pi == 2 and cp == CT // 2 - 1),
                                perf_mode=DR)
                    nc.scalar.copy(
                        qT_sb[:, ot, nch * 512:(nch + 1) * 512], ps)
            # ================= stage C: pe_attn^T = sigmoid(p2^T p1) =====
            with tc.tile_pool(name="ps_z", bufs=2, space="PSUM") as ps_z:
                for mt in range(NT):
                    zps = ps_z.tile([128, 2, 512], F32, tag="z")
                    for nq in range(NQ // 512):
                        for ct in range(CT):
                            nc.tensor.matmul(
                                zps[:, nq],
                                p2_sb[:, ct, mt * 128:(mt + 1) * 128],
                                p1_sb[:, ct, nq * 512:(nq + 1) * 512],
                                start=(ct == 0), stop=(ct == CT - 1))
                    nc.scalar.activation(pa[:, mt], zps, AF.Sigmoid)

            for nt in range(NT):
                ps = ps_mm.tile([128, 512], F32, tag="mm")
                for pi, (wi, ai) in enumerate([(0, 0), (0, 1), (1, 0)]):
                    for cp in range(CT // 2):
                        nc.tensor.matmul(
                            ps,
                            x_sb[ai][:, 2 * cp:2 * cp + 2,
                                     nt * 128:(nt + 1) * 128],
                            qw_sb[wi][:, 2 * cp:2 * cp + 2, 2 * C:3 * C],
                            start=(pi == 0 and cp == 0),
                            stop=(pi == 2 and cp == CT // 2 - 1),
                            perf_mode=DR)
                nc.vector.tensor_copy(v_sb[:, nt, :, 0:D],
                                      ps.rearrange("p (h d) -> p h d", h=H))

        # ================= stage E: attention =================
        ps_abcd.__exit__(None, None, None)
        out_pool = ctx.enter_context(tc.tile_pool(name="outp", bufs=1))
        t2_pool = ctx.enter_context(tc.tile_pool(name="t2p", bufs=4))
        e2_pool = ctx.enter_context(tc.tile_pool(name="e2p", bufs=4))
        o_sb = out_pool.tile([128, CT, NQ], BF16)

        ps_e = ExitStack()
        ps_t2 = ps_e.enter_context(tc.tile_pool(name="ps_t2", bufs=3,
                                                space="PSUM"))
        ps_u = ps_e.enter_context(tc.tile_pool(name="ps_u", bufs=2,
                                               space="PSUM"))

        fin = out_pool.tile([128, CT, NQ], F32)
        for nq in range(NQ // 512):
            for h in range(H):
                row0 = (h % 2) * 64
                kt = h // 2
                pool_path = POOL_PATH_A if (h + nq) % 2 else POOL_PATH_B
                u = ps_u.tile([VW, 512], F32, tag="u")
                for mt4 in range(NT // 4):
                    # two mt2 pairs -> one staged [128, 4, 512] bf16 tile,
                    # one exp instruction, four u-steps
                    t2sb = t2_pool.tile([128, 4, 512], BF16, tag="t2sb")
                    e2 = e2_pool.tile([128, 4, 512], BF16, tag="e2")
                    for half in range(2):
                        mt2 = 2 * mt4 + half
                        t2ps = ps_t2.tile([128, 2, 512], F32, tag="t2")
                        for j in range(2):
                            mt = 2 * mt2 + j
                            nc.tensor.matmul(
                                t2ps[:, j],
                                kT_sb[row0:row0 + 64, kt,
                                      mt * 128:(mt + 1) * 128],
                                qT_sb[row0:row0 + 64, kt,
                                      nq * 512:(nq + 1) * 512],
                                start=True, stop=True)
                        pa_sl = pa[:, 2 * mt2:2 * mt2 + 2,
                                   nq * 512:(nq + 1) * 512]
                        if mt2 % 8 in pool_path:
                            c2 = t2_pool.tile([128, 2, 512], BF16, tag="c2")
                            nc.scalar.copy(c2, t2ps)
                            nc.gpsimd.tensor_mul(
                                t2sb[:, 2 * half:2 * half + 2], c2, pa_sl)
                        else:
                            nc.vector.tensor_mul(
                                t2sb[:, 2 * half:2 * half + 2], t2ps, pa_sl)
                    nc.scalar.activation(e2, t2sb, AF.Exp,
                                         scale=SCALE / (WS * WS))
                    for j in range(4):
                        mt = 4 * mt4 + j
                        nc.tensor.matmul(
                            u, v_sb[:, mt, h, :], e2[:, j],
                            start=(mt == 0), stop=(mt == NT - 1))
                # o = u[0:D] / den, den broadcast via tiny ones-matmul
                rec = work.tile([1, 512], BF16, tag="rec")
                with nc.allow_low_precision(reason="1/den fits bf16"):
                    nc.vector.reciprocal(rec, u[D:D + 1])
                bc = work.tile([D, 512], BF16, tag="recbc")
                nc.gpsimd.partition_broadcast(bc, rec)
                nc.vector.tensor_mul(
                    o_sb[row0:row0 + 64, kt, nq * 512:(nq + 1) * 512],
                    u[0:D], bc)

            # ---- proj for this query chunk (stage F, inline)
            for ot in range(CT):
                ps = ps_u.tile([128, 512], F32, tag="u")
                for ct in range(CT):
                    nc.tensor.matmul(
                        ps, pw_sb[:, ct, ot * 128:(ot + 1) * 128],
                        o_sb[:, ct, nq * 512:(nq + 1) * 512],
                        start=(ct == 0), stop=(ct == CT - 1))
                nc.vector.tensor_scalar_add(
                    fin[:, ot, nq * 512:(nq + 1) * 512], ps,
                    pbias[:, ot:ot + 1])
                dma_engs[ot % 3].dma_start(
                    outT.rearrange("(t p) n -> p t n",
                                   p=128)[:, ot, nq * 512:(nq + 1) * 512],
                    fin[:, ot, nq * 512:(nq + 1) * 512])
        ps_e.close()


_NC_CACHE = {}


def _get_nc():
    if "nc" not in _NC_CACHE:
        _NC_CACHE["nc"] = build()
    return _NC_CACHE["nc"]


def make_in_maps(x, pe, qkv_w, proj_w, proj_b, conv1_w, conv1_b, gn1_g, gn1_b,
                 conv2_w, conv2_b, gn2_g, gn2_b):
    f = np.float32
    f8 = ml_dtypes.float8_e4m3
    bf = ml_dtypes.bfloat16
    def pair8(a):
        a8 = a.astype(f8)
        ar = (a - a8.astype(f)).astype(f8)
        return a8, ar

    cw1p = pair8(np.asarray(conv1_w, f).T * WS)
    cw2p = pair8(np.asarray(conv2_w, f).T * WS)
    qwp = pair8(np.asarray(qkv_w, f).T * WS)
    shared = {
        "cw1_0": cw1p[0], "cw1_1": cw1p[1],
        "cw2_0": cw2p[0], "cw2_1": cw2p[1],
        "qw0": qwp[0], "qw1": qwp[1],
        "pw": np.asarray(proj_w, f).T.astype(bf),
        "cb1": np.asarray(conv1_b, f) * WS,
        "cb2": np.asarray(conv2_b, f) * WS,
        "gn1g": np.asarray(gn1_g, f),
        "gn1b": np.asarray(gn1_b, f),
        "gn2g": np.asarray(gn2_g, f),
        "gn2b": np.asarray(gn2_b, f),
        "pb": np.asarray(proj_b, f),
        "gmask": np.repeat(np.eye(2, dtype=f), 64, axis=0),
        "gmaskT": np.ascontiguousarray(np.repeat(np.eye(2, dtype=f), 64,
                                                 axis=0).T),
    }
    shared = {k: np.ascontiguousarray(v) for k, v in shared.items()}
    in_maps = []
    for c in range(N_CORES):
        b, h = c // 2, c % 2
        xT = np.asarray(x[b], f).T
        peT = np.asarray(pe[b], f).T
        if h == 1:
            xT = np.concatenate([xT[:, NQ:], xT[:, :NQ]], axis=1)
            peT = np.concatenate([peT[:, NQ:], peT[:, :NQ]], axis=1)
        m = dict(shared)
        xp = pair8(xT)
        pep = pair8(peT)
        m["xT0"], m["xT1"] = xp
        m["peT0"], m["peT1"] = pep
        in_maps.append(m)
    return in_maps


def assemble_out(results):
    B = N_CORES // 2
    out = np.empty((B, N, C), np.float32)
    for c in range(N_CORES):
        b, h = c // 2, c % 2
        out[b, h * NQ:(h + 1) * NQ, :] = results[c]["outT"].T
    return out


def kernel(**inputs):
    nc = _get_nc()
    in_maps = make_in_maps(**inputs)
    r = run_bass_kernel_spmd(nc, in_maps, core_ids=list(range(N_CORES)))
    return assemble_out(r.results)


if __name__ == "__main__":
    nc = build()
    print("build+compile OK")


# revision 54
# speedup vs baseline: 1.1667x; 1.0745x over previous
"""TRN2 Bass kernel for nn_Attention_87308095193383.

Sharding: 8 cores = (batch b in 0..3) x (query-half h in 0..1).
Each core computes, for its batch:
  - conv1/conv2 + GroupNorm fully (stats need full N),
  - pe_attn^T slice [m=2048, n=1024] for its query half,
  - k,v fully; q for its half,
  - attention + proj for its half -> outT [512, 1024].
Host permutes the N columns per core so "my queries" are always columns
0:1024 of the device program (SPMD: one program, per-core data).

Perf notes (cost model: matmul time = out_free * pe_cycle * cyc_per_row):
  - conv and qkv matmuls run as fp8e4m3 error-feedback pairs (A ~ A8+Ar,
    host-split) with DoubleRow perf mode: 3 passes of 0.5 cyc/row beat
    one bf16 pass (1 cyc/row) at the same ~2e-3 accuracy. z/s/u/proj
    stay bf16: fp8 on the attention chain fails the 2e-2 gate because
    the output is a near-cancelling weighted average (quantization noise
    does not shrink relative to the sqrt(N)-suppressed signal).
  - Weights are scaled x16 on host so fp8 values sit in the normal range;
    GroupNorm is scale-invariant (eps -> 256*eps, conv bias x16), the
    qk logits pick up x256 folded into the exp scale, and v's x16 cancels
    against a 16-valued denominator column (VW=66: DoubleRow-legal even
    width).
  - Engine split (GPSIMD cannot touch PSUM on real hw; DMA cannot read
    PSUM): exp/sigmoid and part of the PSUM evacuation on Act; s*pa muls
    mostly DVE (only engine with PSUM TensorTensor); 3/8 of muls route
    via an Act PSUM->SBUF copy so Pool can legally multiply in SBUF.
  - Attention streams are nq-major with proj inlined per query chunk so
    the projection tail hides inside the attention phase.
"""
import numpy as np
import ml_dtypes

import concourse.bass as bass
import concourse.mybir as mybir
import concourse.tile as tile
from concourse import bacc
from concourse.bass_utils import run_bass_kernel_spmd

F32R = mybir.dt.float32r
F32 = mybir.dt.float32
BF16 = mybir.dt.bfloat16
F8 = mybir.dt.float8e4
AF = mybir.ActivationFunctionType
ALU = mybir.AluOpType
DR = mybir.MatmulPerfMode.DoubleRow

N_CORES = 8
C = 512          # channels
CT = C // 128    # 4 c-tiles
N = 2048         # sequence length
NT = N // 128    # 16 m-tiles
NQ = 1024        # queries per core
H = 8            # heads
D = 64           # head dim
SCALE = D ** -0.5
EPS = 1e-5
WS = 16.0        # host weight scale (fp8 range)
GROUPS = 8       # 2 groups per 128-partition tile (64 ch/group)

# mt2 slots routed via Act copy (PSUM->SBUF bf16) + Pool mul (SBUF only;
# GPSIMD cannot access PSUM on real hw). Other slots: DVE mul from PSUM.
POOL_PATH_A = (2,)
POOL_PATH_B = (2, 6)
VW = 66          # v row width: 64 d + denominator col + pad (DR needs even)


def build():
    nc = bacc.Bacc("TRN2", target_bir_lowering=False, debug=False,
                   num_devices=N_CORES)

    def din(name, shape, dt):
        return nc.dram_tensor(name, shape, dt, kind="ExternalInput").ap()

    peT = [din(f"peT{i}", [C, N], F8) for i in range(2)]
    xT = [din(f"xT{i}", [C, N], F8) for i in range(2)]
    cw1 = [din(f"cw1_{i}", [C, C], F8) for i in range(2)]   # conv1_w.T*16
    cw2 = [din(f"cw2_{i}", [C, C], F8) for i in range(2)]
    qw = [din(f"qw{i}", [C, 3 * C], F8) for i in range(2)]  # qkv_w.T*16
    pw = din("pw", [C, C], BF16)    # proj_w.T
    cb1 = din("cb1", [C], F32)      # conv1_b * 16
    cb2 = din("cb2", [C], F32)
    gn1g = din("gn1g", [C], F32)
    gn1b = din("gn1b", [C], F32)
    gn2g = din("gn2g", [C], F32)
    gn2b = din("gn2b", [C], F32)
    pb = din("pb", [C], F32)
    gmask_in = din("gmask", [128, 2], F32)
    gmaskT_in = din("gmaskT", [2, 128], F32)
    outT = nc.dram_tensor("outT", [C, NQ], F32, kind="ExternalOutput").ap()

    with tile.TileContext(nc) as tc:
        _build_body(nc, tc, peT, xT, cw1, cw2, qw, pw, cb1, cb2,
                    gn1g, gn1b, gn2g, gn2b, pb, gmask_in, gmaskT_in, outT)
    nc.compile()
    return nc


def _build_body(nc, tc, peT, xT, cw1, cw2, qw, pw, cb1, cb2,
                gn1g, gn1b, gn2g, gn2b, pb, gmask_in, gmaskT_in, outT):
    from contextlib import ExitStack
    ctx = ExitStack()
    with ctx:
        consts = ctx.enter_context(tc.tile_pool(name="consts", bufs=1))
        work = ctx.enter_context(tc.tile_pool(name="work", bufs=3))
        dma_engs = (nc.sync, nc.scalar, nc.gpsimd)

        # ---- prefetch every DRAM input up front (DMA idles later anyway)
        in_pool = ctx.enter_context(tc.tile_pool(name="inp", bufs=1))
        pe_sb = [in_pool.tile([128, CT, N], F8, name=f"pe_sb{i}")
                 for i in range(2)]
        x_sb = [in_pool.tile([128, CT, N], F8, name=f"x_sb{i}")
                 for i in range(2)]
        qw_sb = [in_pool.tile([128, CT, 3 * C], F8, name=f"qw_sb{i}")
                 for i in range(2)]
        cw1_sb = [in_pool.tile([128, CT, C], F8, name=f"cw1_sb{i}")
                  for i in range(2)]
        cw2_sb = [in_pool.tile([128, CT, C], F8, name=f"cw2_sb{i}")
                  for i in range(2)]
        # conv inputs (pe + cw) first so conv matmuls start ASAP; x/qw
        # (needed only by qkv, ~40us later) queue behind them.
        for i in range(2):
            pe_r = peT[i].rearrange("(t p) n -> p t n", p=128)
            for ct in range(CT):
                dma_engs[(ct + i) % 3].dma_start(pe_sb[i][:, ct], pe_r[:, ct])
            nc.gpsimd.dma_start(cw1_sb[i],
                                cw1[i].rearrange("(t p) o -> p t o", p=128))
            nc.scalar.dma_start(cw2_sb[i],
                                cw2[i].rearrange("(t p) o -> p t o", p=128))
        for i in range(2):
            x_r = xT[i].rearrange("(t p) n -> p t n", p=128)
            qw_r = qw[i].rearrange("(t p) o -> p t o", p=128)
            for ct in range(CT):
                dma_engs[(ct + 1 + i) % 3].dma_start(x_sb[i][:, ct], x_r[:, ct])
                dma_engs[(ct + 2 + i) % 3].dma_start(qw_sb[i][:, ct], qw_r[:, ct])
        pw_sb = in_pool.tile([128, CT, C], BF16)
        nc.sync.dma_start(pw_sb, pw.rearrange("(t p) o -> p t o", p=128))

        # ---- constants
        gmask = consts.tile([128, 2], F32)     # group-membership mask
        nc.sync.dma_start(gmask, gmask_in)
        gmaskT = consts.tile([2, 128], F32)
        nc.sync.dma_start(gmaskT, gmaskT_in)
        epst = consts.tile([128, 1], F32)
        nc.vector.memset(epst, EPS * WS * WS)   # eps in the x16 domain

        bias1 = consts.tile([128, CT], F32)
        nc.sync.dma_start(bias1, cb1.rearrange("(t p) -> p t", p=128))
        bias2 = consts.tile([128, CT], F32)
        nc.sync.dma_start(bias2, cb2.rearrange("(t p) -> p t", p=128))
        g1g = consts.tile([128, CT], F32)
        nc.sync.dma_start(g1g, gn1g.rearrange("(t p) -> p t", p=128))
        g1b = consts.tile([128, CT], F32)
        nc.sync.dma_start(g1b, gn1b.rearrange("(t p) -> p t", p=128))
        g2g = consts.tile([128, CT], F32)
        nc.sync.dma_start(g2g, gn2g.rearrange("(t p) -> p t", p=128))
        g2b = consts.tile([128, CT], F32)
        nc.sync.dma_start(g2b, gn2b.rearrange("(t p) -> p t", p=128))
        pbias = consts.tile([128, CT], F32)
        nc.sync.dma_start(pbias, pb.rearrange("(t p) -> p t", p=128))

        # ---- persistent activations
        pa_pool = ctx.enter_context(tc.tile_pool(name="pa", bufs=1))
        pa = pa_pool.tile([128, NT, NQ], BF16)     # sigmoid(pe_attn)^T tiles
        kqv_pool = ctx.enter_context(tc.tile_pool(name="kqv", bufs=1))
        kT_sb = kqv_pool.tile([128, CT, N], BF16)
        qT_sb = kqv_pool.tile([128, CT, NQ], BF16)
        v_sb = kqv_pool.tile([128, NT, H, VW], BF16)
        # denominator column: v's x16 scale cancels against this 16.
        nc.vector.memset(v_sb[:, :, :, D:VW], WS)

        ps_abcd = tc.tile_pool(name="ps_mm", bufs=4, space="PSUM")
        ps_mm = ps_abcd.__enter__()

        # ================= stage A/B: conv + groupnorm =================
        with tc.tile_pool(name="p12", bufs=1) as p12_pool:
            # fp8 staging, GN affine applied in place (scale-preserving).
            p1_sb = p12_pool.tile([128, CT, NQ], F8)
            p2_sb = p12_pool.tile([128, CT, N], F8)

            for conv_i, (cwsb, cbt, gg, gb, dst8, keep) in enumerate([
                    (cw1_sb, bias1, g1g, g1b, p1_sb, NQ),
                    (cw2_sb, bias2, g2g, g2b, p2_sb, N)]):
                stats = work.tile([128, CT, N // 512, 6], F32, tag="gnstats")
                mv2 = work.tile([128, 2, CT], F32, tag="gnmv")
                stack3 = work.tile([128, 3, CT], F32, tag="gnstack")
                for ot in range(CT):
                    for nch in range(N // 512):
                        ps = ps_mm.tile([128, 512], F32, tag="mm")
                        passes = [(0, 0), (0, 1), (1, 0)]
                        for pi, (wi, ai) in enumerate(passes):
                            for cp in range(CT // 2):
                                nc.tensor.matmul(
                                    ps,
                                    cwsb[wi][:, 2 * cp:2 * cp + 2,
                                             ot * 128:(ot + 1) * 128],
                                    pe_sb[ai][:, 2 * cp:2 * cp + 2,
                                              nch * 512:(nch + 1) * 512],
                                    start=(pi == 0 and cp == 0),
                                    stop=(pi == 2 and cp == CT // 2 - 1),
                                    perf_mode=DR)
                        nc.vector.bn_stats(stats[:, ot, nch], ps)
                        if nch * 512 < keep:
                            nc.scalar.copy(
                                dst8[:, ot, nch * 512:(nch + 1) * 512], ps)
                    nc.vector.bn_aggr(mv2[:, :, ot], stats[:, ot])
                nc.vector.tensor_add(stack3[:, 0], mv2[:, 0], cbt)
                nc.vector.tensor_copy(stack3[:, 1], mv2[:, 1])
                nc.vector.tensor_mul(stack3[:, 2], stack3[:, 0], stack3[:, 0])
                # group sums over 64-partition halves (all ots at once, N=12)
                gs = ps_mm.tile([2, 3, CT], F32, tag="mm")
                nc.tensor.matmul(gs, gmask, stack3.rearrange("p a t -> p (a t)"),
                                 start=True, stop=True)
                gss = work.tile([2, 3, CT], F32, tag="gss")
                nc.scalar.copy(gss, gs)
                gstat = work.tile([2, 2, CT], F32, tag="gstat")  # [mean, rstd]
                nc.vector.tensor_scalar_mul(gstat[:, 0], gss[:, 0], 1.0 / 64.0)
                vt = work.tile([2, 2, CT], F32, tag="gvtmp")
                nc.vector.tensor_add(vt[:, 0], gss[:, 1], gss[:, 2])
                nc.vector.tensor_scalar_mul(vt[:, 0], vt[:, 0], 1.0 / 64.0)
                nc.vector.tensor_mul(vt[:, 1], gstat[:, 0], gstat[:, 0])
                nc.vector.tensor_sub(vt[:, 0], vt[:, 0], vt[:, 1])
                nc.scalar.activation(vt[:, 0], vt[:, 0], AF.Sqrt, bias=epst[0:2])
                nc.vector.reciprocal(gstat[:, 1], vt[:, 0])
                # broadcast group [mean, rstd] to partitions via indicator MM
                bc_ps = ps_mm.tile([128, 2, CT], F32, tag="mm")
                nc.tensor.matmul(bc_ps, gmaskT,
                                 gstat.rearrange("p a t -> p (a t)"),
                                 start=True, stop=True)
                bcst = work.tile([128, 2, CT], F32, tag="gbc")
                nc.scalar.copy(bcst, bc_ps)
                # per-channel affine: y = x*sc + sh  (bf16 staging -> fp8)
                sc = work.tile([128, 2, CT], F32, tag="gsc")
                nc.vector.tensor_mul(sc[:, 0], bcst[:, 1], gg)
                nc.vector.tensor_sub(sc[:, 1], cbt, bcst[:, 0])
                nc.vector.tensor_mul(sc[:, 1], sc[:, 1], sc[:, 0])
                nc.vector.tensor_add(sc[:, 1], sc[:, 1], gb)
                for ot in range(CT):
                    aeng = nc.vector if (conv_i == 1 and ot % 2) else nc.gpsimd
                    aeng.tensor_scalar(
                        dst8[:, ot, 0:keep], dst8[:, ot, 0:keep],
                        sc[:, 0, ot:ot + 1], sc[:, 1, ot:ot + 1],
                        op0=ALU.mult, op1=ALU.add)

            # ================= stage D: qkv =================
            # kT (full N) and qT (first NQ) in bf16; v in fp8 (x16)
            for ot in range(CT):
                for nch in range(N // 512):
                    ps = ps_mm.tile([128, 512], F32, tag="mm")
                    for pi, (wi, ai) in enumerate([(0, 0), (0, 1), (1, 0)]):
                        for cp in range(CT // 2):
                            nc.tensor.matmul(
                                ps,
                                qw_sb[wi][:, 2 * cp:2 * cp + 2,
                                          C + ot * 128:C + (ot + 1) * 128],
                                x_sb[ai][:, 2 * cp:2 * cp + 2,
                                         nch * 512:(nch + 1) * 512],
                                start=(pi == 0 and cp == 0),
                                stop=(pi == 2 and cp == CT // 2 - 1),
                                perf_mode=DR)
                    keng = nc.scalar if nch % 2 else nc.vector
                    (keng.copy if nch % 2 else keng.tensor_copy)(
                        kT_sb[:, ot, nch * 512:(nch + 1) * 512], ps)
                for nch in range(NQ // 512):
                    ps = ps_mm.tile([128, 512], F32, tag="mm")
                    for pi, (wi, ai) in enumerate([(0, 0), (0, 1), (1, 0)]):
                        for cp in range(CT // 2):
                            nc.tensor.matmul(
                                ps,
                                qw_sb[wi][:, 2 * cp:2 * cp + 2,
                                          ot * 128:(ot + 1) * 128],
                                x_sb[ai][:, 2 * cp:2 * cp + 2,
                                         nch * 512:(nch + 1) * 512],
                                start=(pi == 0 and cp == 0),
                                stop=(pi == 2 and cp == CT // 2 - 1),
                                perf_mode=DR)
                    nc.scalar.copy(
                        qT_sb[:, ot, nch * 512:(nch + 1) * 512], ps)
            # ================= stage C: pe_attn^T = sigmoid(p2^T p1) =====
            with tc.tile_pool(name="ps_z", bufs=2, space="PSUM") as ps_z:
                for mt in range(NT):
                    zps = ps_z.tile([128, 2, 512], F32, tag="z")
                    for nq in range(NQ // 512):
                        for cp in range(CT // 2):
                            nc.tensor.matmul(
                                zps[:, nq],
                                p2_sb[:, 2 * cp:2 * cp + 2,
                                      mt * 128:(mt + 1) * 128],
                                p1_sb[:, 2 * cp:2 * cp + 2,
                                      nq * 512:(nq + 1) * 512],
                                start=(cp == 0), stop=(cp == CT // 2 - 1),
                                perf_mode=DR)
                    nc.scalar.activation(pa[:, mt], zps, AF.Sigmoid)

            for nt in range(NT):
                ps = ps_mm.tile([128, 512], F32, tag="mm")
                for pi, (wi, ai) in enumerate([(0, 0), (0, 1), (1, 0)]):
                    for cp in range(CT // 2):
                        nc.tensor.matmul(
                            ps,
                            x_sb[ai][:, 2 * cp:2 * cp + 2,
                                     nt * 128:(nt + 1) * 128],
                            qw_sb[wi][:, 2 * cp:2 * cp + 2, 2 * C:3 * C],
                            start=(pi == 0 and cp == 0),
                            stop=(pi == 2 and cp == CT // 2 - 1),
                            perf_mode=DR)
                nc.vector.tensor_copy(v_sb[:, nt, :, 0:D],
                                      ps.rearrange("p (h d) -> p h d", h=H))

        # ================= stage E: attention =================
        ps_abcd.__exit__(None, None, None)
        out_pool = ctx.enter_context(tc.tile_pool(name="outp", bufs=1))
        t2_pool = ctx.enter_context(tc.tile_pool(name="t2p", bufs=4))
        e2_pool = ctx.enter_context(tc.tile_pool(name="e2p", bufs=4))
        o_sb = out_pool.tile([128, CT, NQ], BF16)

        ps_e = ExitStack()
        ps_t2 = ps_e.enter_context(tc.tile_pool(name="ps_t2", bufs=3,
                                                space="PSUM"))
        ps_u = ps_e.enter_context(tc.tile_pool(name="ps_u", bufs=2,
                                               space="PSUM"))

        fin = out_pool.tile([128, CT, NQ], F32)
        for nq in range(NQ // 512):
            for h in range(H):
                row0 = (h % 2) * 64
                kt = h // 2
                pool_path = POOL_PATH_A if (h + nq) % 2 else POOL_PATH_B
                u = ps_u.tile([VW, 512], F32, tag="u")
                for mt4 in range(NT // 4):
                    # two mt2 pairs -> one staged [128, 4, 512] bf16 tile,
                    # one exp instruction, four u-steps
                    t2sb = t2_pool.tile([128, 4, 512], BF16, tag="t2sb")
                    e2 = e2_pool.tile([128, 4, 512], BF16, tag="e2")
                    for half in range(2):
                        mt2 = 2 * mt4 + half
                        t2ps = ps_t2.tile([128, 2, 512], F32, tag="t2")
                        for j in range(2):
                            mt = 2 * mt2 + j
                            nc.tensor.matmul(
                                t2ps[:, j],
                                kT_sb[row0:row0 + 64, kt,
                                      mt * 128:(mt + 1) * 128],
                                qT_sb[row0:row0 + 64, kt,
                                      nq * 512:(nq + 1) * 512],
                                start=True, stop=True)
                        pa_sl = pa[:, 2 * mt2:2 * mt2 + 2,
                                   nq * 512:(nq + 1) * 512]
                        if mt2 % 8 in pool_path:
                            c2 = t2_pool.tile([128, 2, 512], BF16, tag="c2")
                            nc.scalar.copy(c2, t2ps)
                            nc.gpsimd.tensor_mul(
                                t2sb[:, 2 * half:2 * half + 2], c2, pa_sl)
                        else:
                            nc.vector.tensor_mul(
                                t2sb[:, 2 * half:2 * half + 2], t2ps, pa_sl)
                    nc.scalar.activation(e2, t2sb, AF.Exp,
                                         scale=SCALE / (WS * WS))
                    for j in range(4):
                        mt = 4 * mt4 + j
                        nc.tensor.matmul(
                            u, v_sb[:, mt, h, :], e2[:, j],
                            start=(mt == 0), stop=(mt == NT - 1))
                # o = u[0:D] / den, den broadcast via tiny ones-matmul
                rec = work.tile([1, 512], BF16, tag="rec")
                with nc.allow_low_precision(reason="1/den fits bf16"):
                    nc.vector.reciprocal(rec, u[D:D + 1])
                bc = work.tile([D, 512], BF16, tag="recbc")
                nc.gpsimd.partition_broadcast(bc, rec)
                nc.vector.tensor_mul(
                    o_sb[row0:row0 + 64, kt, nq * 512:(nq + 1) * 512],
                    u[0:D], bc)

            # ---- proj for this query chunk (stage F, inline)
            for ot in range(CT):
                ps = ps_u.tile([128, 512], F32, tag="u")
                for ct in range(CT):
                    nc.tensor.matmul(
                        ps, pw_sb[:, ct, ot * 128:(ot + 1) * 128],
                        o_sb[:, ct, nq * 512:(nq + 1) * 512],
                        start=(ct == 0), stop=(ct == CT - 1))
                nc.vector.tensor_scalar_add(
                    fin[:, ot, nq * 512:(nq + 1) * 512], ps,
                    pbias[:, ot:ot + 1])
                dma_engs[ot % 3].dma_start(
                    outT.rearrange("(t p) n -> p t n",
                                   p=128)[:, ot, nq * 512:(nq + 1) * 512],
                    fin[:, ot, nq * 512:(nq + 1) * 512])
        ps_e.close()


_NC_CACHE = {}


def _get_nc():
    if "nc" not in _NC_CACHE:
        _NC_CACHE["nc"] = build()
    return _NC_CACHE["nc"]


def make_in_maps(x, pe, qkv_w, proj_w, proj_b, conv1_w, conv1_b, gn1_g, gn1_b,
                 conv2_w, conv2_b, gn2_g, gn2_b):
    f = np.float32
    f8 = ml_dtypes.float8_e4m3
    bf = ml_dtypes.bfloat16
    def pair8(a):
        a8 = a.astype(f8)
        ar = (a - a8.astype(f)).astype(f8)
        return a8, ar

    cw1p = pair8(np.asarray(conv1_w, f).T * WS)
    cw2p = pair8(np.asarray(conv2_w, f).T * WS)
    qwp = pair8(np.asarray(qkv_w, f).T * WS)
    shared = {
        "cw1_0": cw1p[0], "cw1_1": cw1p[1],
        "cw2_0": cw2p[0], "cw2_1": cw2p[1],
        "qw0": qwp[0], "qw1": qwp[1],
        "pw": np.asarray(proj_w, f).T.astype(bf),
        "cb1": np.asarray(conv1_b, f) * WS,
        "cb2": np.asarray(conv2_b, f) * WS,
        "gn1g": np.asarray(gn1_g, f),
        "gn1b": np.asarray(gn1_b, f),
        "gn2g": np.asarray(gn2_g, f),
        "gn2b": np.asarray(gn2_b, f),
        "pb": np.asarray(proj_b, f),
        "gmask": np.repeat(np.eye(2, dtype=f), 64, axis=0),
        "gmaskT": np.ascontiguousarray(np.repeat(np.eye(2, dtype=f), 64,
                                                 axis=0).T),
    }
    shared = {k: np.ascontiguousarray(v) for k, v in shared.items()}
    in_maps = []
    for c in range(N_CORES):
        b, h = c // 2, c % 2
        xT = np.asarray(x[b], f).T
        peT = np.asarray(pe[b], f).T
        if h == 1:
            xT = np.concatenate([xT[:, NQ:], xT[:, :NQ]], axis=1)
            peT = np.concatenate([peT[:, NQ:], peT[:, :NQ]], axis=1)
        m = dict(shared)
        xp = pair8(xT)
        pep = pair8(peT)
        m["xT0"], m["xT1"] = xp
        m["peT0"], m["peT1"] = pep
        in_maps.append(m)
    return in_maps


def assemble_out(results):
    B = N_CORES // 2
    out = np.empty((B, N, C), np.float32)
    for c in range(N_CORES):
        b, h = c // 2, c % 2
        out[b, h * NQ:(h + 1) * NQ, :] = results[c]["outT"].T
    return out


def kernel(**inputs):
    nc = _get_nc()
    in_maps = make_in_maps(**inputs)
    r = run_bass_kernel_spmd(nc, in_maps, core_ids=list(range(N_CORES)))
    return assemble_out(r.results)


if __name__ == "__main__":
    nc = build()
    print("build+compile OK")
